# revision 14
# baseline (speedup 1.0000x reference)
"""Trainium2 kernel for nn_IteratedLinearNet: y = x @ (W.T)^60.

Strategy (8 NeuronCores, single SPMD launch):
  - A' = c*W.T with c = 1/spectral_radius(W.T) estimated host-side, so all
    powers of A' stay O(1) and the whole chain runs in float16 at full PE
    rate with half the DMA/collective bytes of fp32; c^-60 undone on host.
  - row-slab matrix-power chain: core j computes C_{a+b}[Sj,:] =
    C_a[Sj,:] @ C_b with the transposed own slab as stationary operand and
    a replicated full C_b.  Chain 2,3,4,8,12,24,36,48: products 3,4 use
    the resident input A' and 36,48 reuse the resident C12, so only C4,
    C12, C48 are gathered -> 3 AllGathers total.
  - the apply is split as y = (x @ C12) @ C48: u^T = C12^T @ x^T runs
    during the C48 AllGather (computed directly in transposed form with
    the resident C12 as stationary), and filler products C5 = C4 @ A',
    C13 = C12 @ A' occupy the other two AllGather windows, so TensorE
    never idles long enough for the HAM clock gate to throttle.
  - gathered matrices stream from DRAM K-chunk by K-chunk on BOTH hwdge
    queues (alternating), so reloads never pace the matmuls.
  - stationary-slab transposes run on TensorE, interleaved chunk-by-chunk
    just ahead of the consuming matmuls.
  - a dummy warmup AllGather absorbs the one-time collective barrier/ncfw
    ramp while the first products (which need no comm) run.

Self-contained: builds/compiles on first call and caches the module.
"""

import numpy as np

_G = 2048
_B = 4096
_NC = 8
_SW = _G // _NC  # 256 slab rows per core
_BW = _B // _NC  # 512 batch rows per core
_KT = _G // 128  # 16 K-chunks
_H = _G // 2

_cache = {}


def _build():
    from contextlib import ExitStack

    import concourse.tile as tile
    from concourse import bacc, masks, mybir

    F16 = mybir.dt.float16
    F32 = mybir.dt.float32
    G, KT, SW, BW = _G, _KT, _SW, _BW

    nc = bacc.Bacc(None, target_bir_lowering=False, num_devices=_NC)
    afull = nc.declare_dram_parameter("afull", [G, G], F16, isOutput=False)
    slabt0 = nc.declare_dram_parameter("slabt0", [G, SW], F16, isOutput=False)
    xbj = nc.declare_dram_parameter("xbj", [G, BW], F16, isOutput=False)
    yj = nc.declare_dram_parameter("yj", [BW, G], F32, isOutput=True)

    rg = [list(range(_NC))]

    with ExitStack() as ctx:
        tc = ctx.enter_context(tile.TileContext(nc))
        fullp = ctx.enter_context(tc.tile_pool(name="fullp", bufs=2))
        stp = ctx.enter_context(tc.tile_pool(name="stp", bufs=2))
        ssp = ctx.enter_context(tc.tile_pool(name="ssp", bufs=4))
        misc = ctx.enter_context(tc.tile_pool(name="misc", bufs=1))
        ypool = ctx.enter_context(tc.tile_pool(name="ypool", bufs=4))
        mmps = ctx.enter_context(tc.tile_pool(name="mmps", bufs=6, space="PSUM"))
        tps = ctx.enter_context(tc.tile_pool(name="tps", bufs=2, space="PSUM"))
        dram = ctx.enter_context(tc.tile_pool(name="dram", bufs=2, space="DRAM"))

        qeng = [nc.sync, nc.scalar]  # two hwdge DMA queues

        ident32 = misc.tile([128, 128], F32, name="ident32", tag="ident32")
        masks.make_identity(nc, ident32[:])
        ident = misc.tile([128, 128], F16, name="ident", tag="ident")
        nc.vector.tensor_copy(ident[:], ident32[:])

        # warmup collective: absorbs the one-time barrier + ncfw ramp while
        # the first products (which need no comm) run.
        wtile = misc.tile([128, 16], F16, name="warm", tag="warm")
        nc.vector.tensor_copy(wtile[:], ident32[:, 0:16])
        win = dram.tile([128, 16], F16, name="warmin", tag="warmin")
        nc.scalar.dma_start(win[:], wtile[:])
        wout = dram.tile(
            [128 * _NC, 16], F16, name="warmout", tag="warmout", addr_space="Shared"
        )
        nc.gpsimd.collective_compute(
            "AllGather",
            mybir.AluOpType.bypass,
            replica_groups=rg,
            ins=[win.opt()],
            outs=[wout.opt()],
        )

        # pre-warm the PE clock before the first real product
        for jj in range(64):
            jt = tps.tile([128, 128], F16, name=f"jw{jj}", tag="psT")
            nc.tensor.transpose(jt[:], ident[:], ident[:])

        # step-1 stationary slab + streamed A' on both queues + x^T slice
        st1 = stp.tile([128, KT, SW], F16, name="st1", tag="st")
        fA = fullp.tile([128, KT, G], F16, name="fA", tag="full")
        for k in range(KT):
            qeng[(k + 1) % 2].dma_start(st1[:, k, :], slabt0[128 * k : 128 * (k + 1), :])
            qeng[k % 2].dma_start(fA[:, k, :], afull[128 * k : 128 * (k + 1), :])
        xsb = misc.tile([128, KT, BW], F16, name="xsb", tag="xsb")
        for k in range(KT):
            qeng[k % 2].dma_start(xsb[:, k, :], xbj[128 * k : 128 * (k + 1), :])

        def product(st, prev_ssbs, F, si, reload_src=None, do_ag=False):
            """Emit one slab product = slab(st) @ F.

            If prev_ssbs is given, st is built chunk-by-chunk from them via
            TensorE transposes interleaved ahead of the consuming matmuls.
            """
            if reload_src is not None:
                q3 = [nc.sync, nc.scalar, nc.gpsimd]
                for k in range(KT):
                    q3[k % 3].dma_start(
                        F[:, k, :], reload_src[128 * k : 128 * (k + 1), :]
                    )
            ssbs = []
            for rnd in range(2):
                pts = [
                    [
                        mmps.tile(
                            [128, 512], F32, name=f"p{si}_{rnd}_{mr}_{mci}", tag="mm"
                        )
                        for mci in range(2)
                    ]
                    for mr in range(2)
                ]
                for k in range(KT):
                    if rnd == 0 and prev_ssbs is not None:
                        src = prev_ssbs[k // 8]
                        off = 128 * (k % 8)
                        for r in range(2):
                            psT = tps.tile(
                                [128, 128], F16, name=f"pt{si}_{k}_{r}", tag="psT"
                            )
                            nc.tensor.transpose(
                                psT[:], src[:, r, off : off + 128], ident[:]
                            )
                            nc.vector.tensor_copy(
                                st[:, k, 128 * r : 128 * (r + 1)], psT[:]
                            )
                    for mr in range(2):
                        for mci in range(2):
                            mc = 2 * rnd + mci
                            nc.tensor.matmul(
                                pts[mr][mci][:],
                                st[:, k, 128 * mr : 128 * (mr + 1)],
                                F[:, k, 512 * mc : 512 * (mc + 1)],
                                start=(k == 0),
                                stop=(k == KT - 1),
                            )
                ssb = ssp.tile([128, 2, _H], F16, name=f"ss{si}_{rnd}", tag="ss")
                for mr in range(2):
                    for mci in range(2):
                        nc.vector.tensor_copy(
                            ssb[:, mr, 512 * mci : 512 * (mci + 1)], pts[mr][mci][:]
                        )
                ssbs.append(ssb)
                if do_ag:
                    if rnd == 0:
                        agin = dram.tile([SW, G], F16, name=f"agin{si}", tag="agin")
                    for r in range(2):
                        nc.scalar.dma_start(
                            agin[128 * r : 128 * (r + 1), _H * rnd : _H * (rnd + 1)],
                            ssb[:, r, :],
                        )
            agout = None
            if do_ag:
                agout = dram.tile(
                    [G, G], F16, name=f"agout{si}", tag="agout", addr_space="Shared"
                )
                nc.gpsimd.collective_compute(
                    "AllGather",
                    mybir.AluOpType.bypass,
                    replica_groups=rg,
                    ins=[agin.opt()],
                    outs=[agout.opt()],
                )
            return ssbs, agout

        def new_st(si):
            return stp.tile([128, KT, SW], F16, name=f"st_{si}", tag="st")

        # chain
        ss, _ = product(st1, None, fA, "c2")
        st2 = new_st("c3")
        ss, _ = product(st2, ss, fA, "c3")
        st3 = new_st("c4")
        ss, ag4 = product(st3, ss, fA, "c4", do_ag=True)

        # filler C5 = C4 @ A' occupies the AG(C4) window and builds st4
        st4 = new_st("c8")
        ss, _ = product(st4, ss, fA, "f5")

        fC4 = fullp.tile([128, KT, G], F16, name="fC4", tag="full")
        ss, _ = product(st4, None, fC4, "c8", reload_src=ag4)
        st8 = new_st("c12")
        ss, ag12 = product(st8, ss, fC4, "c12", do_ag=True)

        # filler C13 = C12 @ A' occupies the AG(C12) window and builds st12
        st12 = new_st("c24")
        ss, _ = product(st12, ss, fA, "f13")

        fC12 = fullp.tile([128, KT, G], F16, name="fC12", tag="full")
        ss, _ = product(st12, None, fC12, "c24", reload_src=ag12)
        st24 = new_st("c36")
        ss, _ = product(st24, ss, fC12, "c36")
        st36 = new_st("c48")
        ss, ag48 = product(st36, ss, fC12, "c48", do_ag=True)

        # u^T = C12^T @ x^T during the AG(C48) window (C12 stationary)
        ut = misc.tile([128, KT, BW], F16, name="ut", tag="ut")
        for m in range(KT):
            pu = mmps.tile([128, BW], F32, name=f"pu{m}", tag="mm")
            for k in range(KT):
                nc.tensor.matmul(
                    pu[:],
                    fC12[:, k, 128 * m : 128 * (m + 1)],
                    xsb[:, k, :],
                    start=(k == 0),
                    stop=(k == KT - 1),
                )
            nc.vector.tensor_copy(ut[:, m, :], pu[:])

        # y[Bj,:] = u[Bj,:] @ C48
        fC48 = fullp.tile([128, KT, G], F16, name="fC48", tag="full")
        for k in range(KT):
            qeng[k % 2].dma_start(fC48[:, k, :], ag48[128 * k : 128 * (k + 1), :])
        for mr in range(4):
            pts = [
                mmps.tile([128, 512], F32, name=f"py{mr}_{mc}", tag="mm")
                for mc in range(4)
            ]
            for k in range(KT):
                for mc in range(4):
                    nc.tensor.matmul(
                        pts[mc][:],
                        ut[:, k, 128 * mr : 128 * (mr + 1)],
                        fC48[:, k, 512 * mc : 512 * (mc + 1)],
                        start=(k == 0),
                        stop=(k == KT - 1),
                    )
            for mc in range(4):
                yt = ypool.tile([128, 512], F32, name=f"yt{mr}_{mc}", tag="yt")
                nc.vector.tensor_copy(yt[:], pts[mc][:])
                qeng[mc % 2].dma_start(
                    yj[128 * mr : 128 * (mr + 1), 512 * mc : 512 * (mc + 1)], yt[:]
                )
    nc.compile()
    return nc


def _prep(x, W):
    """Host prep: rescale so the fp16 chain stays O(1); fp16 casts."""
    A = np.ascontiguousarray(W.T.astype(np.float64))
    rng = np.random.default_rng(0)
    v = rng.standard_normal(_G)
    growth = []
    for _ in range(60):
        v2 = A @ v
        n2 = np.linalg.norm(v2)
        growth.append(n2 / np.linalg.norm(v))
        v = v2 / n2
    rho = float(np.exp(np.mean(np.log(growth[20:]))))
    c = 1.0 / rho
    a16 = np.ascontiguousarray((c * A).astype(np.float16))
    x16t = np.ascontiguousarray(x.astype(np.float16).T)
    return a16, x16t, c


def kernel(x, W):
    from concourse.bass_utils import run_bass_kernel_spmd

    if "nc" not in _cache:
        _cache["nc"] = _build()
    nc = _cache["nc"]

    x = np.asarray(x, dtype=np.float32)
    W = np.asarray(W, dtype=np.float32)
    a16, x16t, c = _prep(x, W)
    in_maps = [
        {
            "afull": a16,
            "slabt0": np.ascontiguousarray(a16[_SW * j : _SW * (j + 1), :].T),
            "xbj": np.ascontiguousarray(x16t[:, _BW * j : _BW * (j + 1)]),
        }
        for j in range(_NC)
    ]
    res = run_bass_kernel_spmd(nc, in_maps, core_ids=list(range(_NC)))
    _cache["last_exec_time_ns"] = res.exec_time_ns
    _cache["last_results"] = res
    scale = np.float64(c) ** -60
    y = np.concatenate(
        [res.results[j]["yj"].astype(np.float64) * scale for j in range(_NC)], axis=0
    ).astype(np.float32)
    return y


# revision 15
# speedup vs baseline: 1.0184x; 1.0184x over previous
"""Trainium2 kernel for nn_IteratedLinearNet: y = x @ (W.T)^60.

Strategy (8 NeuronCores, single SPMD launch):
  - A' = c*W.T with c = 1/spectral_radius(W.T) estimated host-side, so all
    powers of A' stay O(1) and the whole chain runs in float16 at full PE
    rate with half the DMA/collective bytes of fp32; c^-60 undone on host.
  - row-slab matrix-power chain: core j computes C_{a+b}[Sj,:] =
    C_a[Sj,:] @ C_b with the transposed own slab as stationary operand and
    a replicated full C_b.  Chain 2,3,4,8,12,24,36,48: products 3,4 use
    the resident input A' and 36,48 reuse the resident C12, so only C4,
    C12, C48 are gathered -> 3 AllGathers total.
  - the apply is split as y = (x @ C12) @ C48: u^T = C12^T @ x^T runs
    during the C48 AllGather (computed directly in transposed form with
    the resident C12 as stationary), and filler products C5 = C4 @ A',
    C13 = C12 @ A' occupy the other two AllGather windows, so TensorE
    never idles long enough for the HAM clock gate to throttle.
  - gathered matrices stream from DRAM K-chunk by K-chunk on BOTH hwdge
    queues (alternating), so reloads never pace the matmuls.
  - stationary-slab transposes run on TensorE, interleaved chunk-by-chunk
    just ahead of the consuming matmuls.
  - a dummy warmup AllGather absorbs the one-time collective barrier/ncfw
    ramp while the first products (which need no comm) run.

Self-contained: builds/compiles on first call and caches the module.
"""

import numpy as np

_G = 2048
_B = 4096
_NC = 8
_SW = _G // _NC  # 256 slab rows per core
_BW = _B // _NC  # 512 batch rows per core
_KT = _G // 128  # 16 K-chunks
_H = _G // 2

_cache = {}


def _build():
    from contextlib import ExitStack

    import concourse.tile as tile
    from concourse import bacc, masks, mybir

    F16 = mybir.dt.float16
    F32 = mybir.dt.float32
    G, KT, SW, BW = _G, _KT, _SW, _BW

    nc = bacc.Bacc(None, target_bir_lowering=False, num_devices=_NC)
    afull = nc.declare_dram_parameter("afull", [G, G], F16, isOutput=False)
    slabt0 = nc.declare_dram_parameter("slabt0", [G, SW], F16, isOutput=False)
    xbj = nc.declare_dram_parameter("xbj", [G, BW], F16, isOutput=False)
    yj = nc.declare_dram_parameter("yj", [BW, G], F32, isOutput=True)

    rg = [list(range(_NC))]

    with ExitStack() as ctx:
        tc = ctx.enter_context(tile.TileContext(nc))
        fullp = ctx.enter_context(tc.tile_pool(name="fullp", bufs=2))
        stp = ctx.enter_context(tc.tile_pool(name="stp", bufs=2))
        ssp = ctx.enter_context(tc.tile_pool(name="ssp", bufs=4))
        misc = ctx.enter_context(tc.tile_pool(name="misc", bufs=1))
        ypool = ctx.enter_context(tc.tile_pool(name="ypool", bufs=4))
        mmps = ctx.enter_context(tc.tile_pool(name="mmps", bufs=6, space="PSUM"))
        tps = ctx.enter_context(tc.tile_pool(name="tps", bufs=2, space="PSUM"))
        dram = ctx.enter_context(tc.tile_pool(name="dram", bufs=2, space="DRAM"))

        qeng = [nc.sync, nc.scalar]  # two hwdge DMA queues

        ident32 = misc.tile([128, 128], F32, name="ident32", tag="ident32")
        masks.make_identity(nc, ident32[:])
        ident = misc.tile([128, 128], F16, name="ident", tag="ident")
        nc.vector.tensor_copy(ident[:], ident32[:])

        # warmup collective: absorbs the one-time barrier + ncfw ramp while
        # the first products (which need no comm) run.
        wtile = misc.tile([128, 16], F16, name="warm", tag="warm")
        nc.vector.tensor_copy(wtile[:], ident32[:, 0:16])
        win = dram.tile([128, 16], F16, name="warmin", tag="warmin")
        nc.scalar.dma_start(win[:], wtile[:])
        wout = dram.tile(
            [128 * _NC, 16], F16, name="warmout", tag="warmout", addr_space="Shared"
        )
        nc.gpsimd.collective_compute(
            "AllGather",
            mybir.AluOpType.bypass,
            replica_groups=rg,
            ins=[win.opt()],
            outs=[wout.opt()],
        )

        # pre-warm the PE clock before the first real product
        for jj in range(160):
            jt = tps.tile([128, 128], F16, name=f"jw{jj}", tag="psT")
            nc.tensor.transpose(jt[:], ident[:], ident[:])

        # step-1 stationary slab + streamed A' on both queues + x^T slice
        st1 = stp.tile([128, KT, SW], F16, name="st1", tag="st")
        fA = fullp.tile([128, KT, G], F16, name="fA", tag="full")
        for k in range(KT):
            qeng[(k + 1) % 2].dma_start(st1[:, k, :], slabt0[128 * k : 128 * (k + 1), :])
            qeng[k % 2].dma_start(fA[:, k, :], afull[128 * k : 128 * (k + 1), :])
        xsb = misc.tile([128, KT, BW], F16, name="xsb", tag="xsb")
        for k in range(KT):
            qeng[k % 2].dma_start(xsb[:, k, :], xbj[128 * k : 128 * (k + 1), :])

        def product(st, prev_ssbs, F, si, reload_src=None, do_ag=False, st_next=None):
            """Emit one slab product = slab(st) @ F.

            If prev_ssbs is given, st chunks 8-15 are built from prev_ssbs[1]
            via TensorE transposes interleaved ahead of the consuming matmuls
            (chunks 0-7 were pre-built by the previous product's rnd1).  If
            st_next is given, its chunks 0-7 are pre-built during rnd1 from
            this product's rnd0 output.
            """
            if reload_src is not None:
                q3 = [nc.sync, nc.scalar, nc.gpsimd]
                for k in range(KT):
                    q3[k % 3].dma_start(
                        F[:, k, :], reload_src[128 * k : 128 * (k + 1), :]
                    )
            ssbs = []
            for rnd in range(2):
                pts = [
                    [
                        mmps.tile(
                            [128, 512], F32, name=f"p{si}_{rnd}_{mr}_{mci}", tag="mm"
                        )
                        for mci in range(2)
                    ]
                    for mr in range(2)
                ]
                for k in range(KT):
                    if rnd == 0 and prev_ssbs is not None and k >= 8:
                        off = 128 * (k % 8)
                        for r in range(2):
                            psT = tps.tile(
                                [128, 128], F16, name=f"pt{si}_{k}_{r}", tag="psT"
                            )
                            nc.tensor.transpose(
                                psT[:], prev_ssbs[1][:, r, off : off + 128], ident[:]
                            )
                            nc.vector.tensor_copy(
                                st[:, k, 128 * r : 128 * (r + 1)], psT[:]
                            )
                    if rnd == 1 and st_next is not None and k < 8:
                        for r in range(2):
                            psT = tps.tile(
                                [128, 128], F16, name=f"pn{si}_{k}_{r}", tag="psT"
                            )
                            nc.tensor.transpose(
                                psT[:], ssbs[0][:, r, 128 * k : 128 * k + 128], ident[:]
                            )
                            nc.vector.tensor_copy(
                                st_next[:, k, 128 * r : 128 * (r + 1)], psT[:]
                            )
                    for mr in range(2):
                        for mci in range(2):
                            mc = 2 * rnd + mci
                            nc.tensor.matmul(
                                pts[mr][mci][:],
                                st[:, k, 128 * mr : 128 * (mr + 1)],
                                F[:, k, 512 * mc : 512 * (mc + 1)],
                                start=(k == 0),
                                stop=(k == KT - 1),
                            )
                ssb = ssp.tile([128, 2, _H], F16, name=f"ss{si}_{rnd}", tag="ss")
                for mr in range(2):
                    for mci in range(2):
                        nc.vector.tensor_copy(
                            ssb[:, mr, 512 * mci : 512 * (mci + 1)], pts[mr][mci][:]
                        )
                ssbs.append(ssb)
                if do_ag:
                    if rnd == 0:
                        agin = dram.tile([SW, G], F16, name=f"agin{si}", tag="agin")
                    for r in range(2):
                        nc.scalar.dma_start(
                            agin[128 * r : 128 * (r + 1), _H * rnd : _H * (rnd + 1)],
                            ssb[:, r, :],
                        )
            agout = None
            if do_ag:
                agout = dram.tile(
                    [G, G], F16, name=f"agout{si}", tag="agout", addr_space="Shared"
                )
                nc.gpsimd.collective_compute(
                    "AllGather",
                    mybir.AluOpType.bypass,
                    replica_groups=rg,
                    ins=[agin.opt()],
                    outs=[agout.opt()],
                )
            return ssbs, agout

        def new_st(si):
            return stp.tile([128, KT, SW], F16, name=f"st_{si}", tag="st")

        # chain
        st2 = new_st("c3")
        ss, _ = product(st1, None, fA, "c2", st_next=st2)
        st3 = new_st("c4")
        ss, _ = product(st2, ss, fA, "c3", st_next=st3)
        st4 = new_st("c8")
        ss, ag4 = product(st3, ss, fA, "c4", do_ag=True, st_next=st4)

        # filler C5 = C4 @ A' occupies the AG(C4) window and finishes st4
        ss, _ = product(st4, ss, fA, "f5")

        fC4 = fullp.tile([128, KT, G], F16, name="fC4", tag="full")
        st8 = new_st("c12")
        ss, _ = product(st4, None, fC4, "c8", reload_src=ag4, st_next=st8)
        st12 = new_st("c24")
        ss, ag12 = product(st8, ss, fC4, "c12", do_ag=True, st_next=st12)

        # filler C13 = C12 @ A' occupies the AG(C12) window and finishes st12
        ss, _ = product(st12, ss, fA, "f13")

        fC12 = fullp.tile([128, KT, G], F16, name="fC12", tag="full")
        st24 = new_st("c36")
        ss, _ = product(st12, None, fC12, "c24", reload_src=ag12, st_next=st24)
        st36 = new_st("c48")
        ss, _ = product(st24, ss, fC12, "c36", st_next=st36)
        ss, ag48 = product(st36, ss, fC12, "c48", do_ag=True)

        # u^T = C12^T @ x^T during the AG(C48) window (C12 stationary)
        ut = misc.tile([128, KT, BW], F16, name="ut", tag="ut")
        for m in range(KT):
            pu = mmps.tile([128, BW], F32, name=f"pu{m}", tag="mm")
            for k in range(KT):
                nc.tensor.matmul(
                    pu[:],
                    fC12[:, k, 128 * m : 128 * (m + 1)],
                    xsb[:, k, :],
                    start=(k == 0),
                    stop=(k == KT - 1),
                )
            nc.vector.tensor_copy(ut[:, m, :], pu[:])

        # y[Bj,:] = u[Bj,:] @ C48
        fC48 = fullp.tile([128, KT, G], F16, name="fC48", tag="full")
        for k in range(KT):
            qeng[k % 2].dma_start(fC48[:, k, :], ag48[128 * k : 128 * (k + 1), :])
        for mr in range(4):
            pts = [
                mmps.tile([128, 512], F32, name=f"py{mr}_{mc}", tag="mm")
                for mc in range(4)
            ]
            for k in range(KT):
                for mc in range(4):
                    nc.tensor.matmul(
                        pts[mc][:],
                        ut[:, k, 128 * mr : 128 * (mr + 1)],
                        fC48[:, k, 512 * mc : 512 * (mc + 1)],
                        start=(k == 0),
                        stop=(k == KT - 1),
                    )
            for mc in range(4):
                yt = ypool.tile([128, 512], F32, name=f"yt{mr}_{mc}", tag="yt")
                nc.vector.tensor_copy(yt[:], pts[mc][:])
                qeng[mc % 2].dma_start(
                    yj[128 * mr : 128 * (mr + 1), 512 * mc : 512 * (mc + 1)], yt[:]
                )
    nc.compile()
    return nc


def _prep(x, W):
    """Host prep: rescale so the fp16 chain stays O(1); fp16 casts."""
    A = np.ascontiguousarray(W.T.astype(np.float64))
    rng = np.random.default_rng(0)
    v = rng.standard_normal(_G)
    growth = []
    for _ in range(60):
        v2 = A @ v
        n2 = np.linalg.norm(v2)
        growth.append(n2 / np.linalg.norm(v))
        v = v2 / n2
    rho = float(np.exp(np.mean(np.log(growth[20:]))))
    c = 1.0 / rho
    a16 = np.ascontiguousarray((c * A).astype(np.float16))
    x16t = np.ascontiguousarray(x.astype(np.float16).T)
    return a16, x16t, c


def kernel(x, W):
    from concourse.bass_utils import run_bass_kernel_spmd

    if "nc" not in _cache:
        _cache["nc"] = _build()
    nc = _cache["nc"]

    x = np.asarray(x, dtype=np.float32)
    W = np.asarray(W, dtype=np.float32)
    a16, x16t, c = _prep(x, W)
    in_maps = [
        {
            "afull": a16,
            "slabt0": np.ascontiguousarray(a16[_SW * j : _SW * (j + 1), :].T),
            "xbj": np.ascontiguousarray(x16t[:, _BW * j : _BW * (j + 1)]),
        }
        for j in range(_NC)
    ]
    res = run_bass_kernel_spmd(nc, in_maps, core_ids=list(range(_NC)))
    _cache["last_exec_time_ns"] = res.exec_time_ns
    _cache["last_results"] = res
    scale = np.float64(c) ** -60
    y = np.concatenate(
        [res.results[j]["yj"].astype(np.float64) * scale for j in range(_NC)], axis=0
    ).astype(np.float32)
    return y


# revision 16
# speedup vs baseline: 1.0412x; 1.0223x over previous
"""Trainium2 kernel for nn_IteratedLinearNet: y = x @ (W.T)^60.

Strategy (8 NeuronCores, single SPMD launch):
  - A' = c*W.T with c = 1/spectral_radius(W.T) estimated host-side, so all
    powers of A' stay O(1) and the whole chain runs in float16 at full PE
    rate with half the DMA/collective bytes of fp32; c^-60 undone on host.
  - row-slab matrix-power chain: core j computes C_{a+b}[Sj,:] =
    C_a[Sj,:] @ C_b with the transposed own slab as stationary operand and
    a replicated full C_b.  Chain 2,3,4,8,12,24,36,48: products 3,4 use
    the resident input A' and 36,48 reuse the resident C12, so only C4,
    C12, C48 are gathered -> 3 AllGathers total.
  - the apply is split as y = (x @ C12) @ C48: u^T = C12^T @ x^T runs
    during the C48 AllGather (computed directly in transposed form with
    the resident C12 as stationary), and filler products C5 = C4 @ A',
    C13 = C12 @ A' occupy the other two AllGather windows, so TensorE
    never idles long enough for the HAM clock gate to throttle.
  - gathered matrices stream from DRAM K-chunk by K-chunk on BOTH hwdge
    queues (alternating), so reloads never pace the matmuls.
  - stationary-slab transposes run on TensorE, interleaved chunk-by-chunk
    just ahead of the consuming matmuls.
  - a dummy warmup AllGather absorbs the one-time collective barrier/ncfw
    ramp while the first products (which need no comm) run.

Self-contained: builds/compiles on first call and caches the module.
"""

import numpy as np

_G = 2048
_B = 4096
_NC = 8
_SW = _G // _NC  # 256 slab rows per core
_BW = _B // _NC  # 512 batch rows per core
_KT = _G // 128  # 16 K-chunks
_H = _G // 2

_cache = {}


def _build():
    from contextlib import ExitStack

    import concourse.tile as tile
    from concourse import bacc, masks, mybir

    F16 = mybir.dt.float16
    F32 = mybir.dt.float32
    G, KT, SW, BW = _G, _KT, _SW, _BW

    nc = bacc.Bacc(None, target_bir_lowering=False, num_devices=_NC)
    afull = nc.declare_dram_parameter("afull", [G, G], F16, isOutput=False)
    slabt0 = nc.declare_dram_parameter("slabt0", [G, SW], F16, isOutput=False)
    xbj = nc.declare_dram_parameter("xbj", [G, BW], F16, isOutput=False)
    yj = nc.declare_dram_parameter("yj", [BW, G], F32, isOutput=True)

    rg = [list(range(_NC))]

    with ExitStack() as ctx:
        tc = ctx.enter_context(tile.TileContext(nc))
        fullp = ctx.enter_context(tc.tile_pool(name="fullp", bufs=2))
        stp = ctx.enter_context(tc.tile_pool(name="stp", bufs=2))
        ssp = ctx.enter_context(tc.tile_pool(name="ssp", bufs=4))
        misc = ctx.enter_context(tc.tile_pool(name="misc", bufs=1))
        ypool = ctx.enter_context(tc.tile_pool(name="ypool", bufs=4))
        mmps = ctx.enter_context(tc.tile_pool(name="mmps", bufs=6, space="PSUM"))
        tps = ctx.enter_context(tc.tile_pool(name="tps", bufs=2, space="PSUM"))
        dram = ctx.enter_context(tc.tile_pool(name="dram", bufs=2, space="DRAM"))

        qeng = [nc.sync, nc.scalar]  # two hwdge DMA queues

        ident32 = misc.tile([128, 128], F32, name="ident32", tag="ident32")
        masks.make_identity(nc, ident32[:])
        ident = misc.tile([128, 128], F16, name="ident", tag="ident")
        nc.vector.tensor_copy(ident[:], ident32[:])

        # warmup collective: absorbs the one-time barrier + ncfw ramp while
        # the first products (which need no comm) run.
        wtile = misc.tile([128, 16], F16, name="warm", tag="warm")
        nc.vector.tensor_copy(wtile[:], ident32[:, 0:16])
        win = dram.tile([128, 16], F16, name="warmin", tag="warmin")
        nc.scalar.dma_start(win[:], wtile[:])
        wout = dram.tile(
            [128 * _NC, 16], F16, name="warmout", tag="warmout", addr_space="Shared"
        )
        nc.gpsimd.collective_compute(
            "AllGather",
            mybir.AluOpType.bypass,
            replica_groups=rg,
            ins=[win.opt()],
            outs=[wout.opt()],
        )

        # pre-warm the PE clock before the first real product
        for jj in range(160):
            jt = tps.tile([128, 128], F16, name=f"jw{jj}", tag="psT")
            nc.tensor.transpose(jt[:], ident[:], ident[:])

        # step-1 stationary slab + streamed A' on both queues + x^T slice
        st1 = stp.tile([128, KT, SW], F16, name="st1", tag="st")
        fA = fullp.tile([128, KT, G], F16, name="fA", tag="full")
        for k in range(KT):
            qeng[(k + 1) % 2].dma_start(st1[:, k, :], slabt0[128 * k : 128 * (k + 1), :])
            qeng[k % 2].dma_start(
                fA[:, k, 0:_H], afull[128 * k : 128 * (k + 1), 0:_H]
            )
        for k in range(KT):
            qeng[k % 2].dma_start(
                fA[:, k, _H:G], afull[128 * k : 128 * (k + 1), _H:G]
            )
        xsb = misc.tile([128, KT, BW], F16, name="xsb", tag="xsb")
        for k in range(KT):
            qeng[k % 2].dma_start(xsb[:, k, :], xbj[128 * k : 128 * (k + 1), :])

        def product(st, prev_ssbs, F, si, reload_src=None, do_ag=False, st_next=None):
            """Emit one slab product = slab(st) @ F.

            If prev_ssbs is given, st chunks 8-15 are built from prev_ssbs[1]
            via TensorE transposes interleaved ahead of the consuming matmuls
            (chunks 0-7 were pre-built by the previous product's rnd1).  If
            st_next is given, its chunks 0-7 are pre-built during rnd1 from
            this product's rnd0 output.
            """
            if reload_src is not None:
                q3 = [nc.sync, nc.scalar, nc.gpsimd]
                for k in range(KT):
                    q3[k % 3].dma_start(
                        F[:, k, :], reload_src[128 * k : 128 * (k + 1), :]
                    )
            ssbs = []
            for rnd in range(2):
                pts = [
                    [
                        mmps.tile(
                            [128, 512], F32, name=f"p{si}_{rnd}_{mr}_{mci}", tag="mm"
                        )
                        for mci in range(2)
                    ]
                    for mr in range(2)
                ]
                for k in range(KT):
                    if rnd == 0 and prev_ssbs is not None and k >= 8:
                        off = 128 * (k % 8)
                        for r in range(2):
                            psT = tps.tile(
                                [128, 128], F16, name=f"pt{si}_{k}_{r}", tag="psT"
                            )
                            nc.tensor.transpose(
                                psT[:], prev_ssbs[1][:, r, off : off + 128], ident[:]
                            )
                            nc.vector.tensor_copy(
                                st[:, k, 128 * r : 128 * (r + 1)], psT[:]
                            )
                    if rnd == 1 and st_next is not None and k < 8:
                        for r in range(2):
                            psT = tps.tile(
                                [128, 128], F16, name=f"pn{si}_{k}_{r}", tag="psT"
                            )
                            nc.tensor.transpose(
                                psT[:], ssbs[0][:, r, 128 * k : 128 * k + 128], ident[:]
                            )
                            nc.vector.tensor_copy(
                                st_next[:, k, 128 * r : 128 * (r + 1)], psT[:]
                            )
                    for mr in range(2):
                        for mci in range(2):
                            mc = 2 * rnd + mci
                            nc.tensor.matmul(
                                pts[mr][mci][:],
                                st[:, k, 128 * mr : 128 * (mr + 1)],
                                F[:, k, 512 * mc : 512 * (mc + 1)],
                                start=(k == 0),
                                stop=(k == KT - 1),
                            )
                ssb = ssp.tile([128, 2, _H], F16, name=f"ss{si}_{rnd}", tag="ss")
                for mr in range(2):
                    for mci in range(2):
                        nc.vector.tensor_copy(
                            ssb[:, mr, 512 * mci : 512 * (mci + 1)], pts[mr][mci][:]
                        )
                ssbs.append(ssb)
                if do_ag:
                    if rnd == 0:
                        agin = dram.tile([SW, G], F16, name=f"agin{si}", tag="agin")
                    for r in range(2):
                        nc.scalar.dma_start(
                            agin[128 * r : 128 * (r + 1), _H * rnd : _H * (rnd + 1)],
                            ssb[:, r, :],
                        )
            agout = None
            if do_ag:
                agout = dram.tile(
                    [G, G], F16, name=f"agout{si}", tag="agout", addr_space="Shared"
                )
                nc.gpsimd.collective_compute(
                    "AllGather",
                    mybir.AluOpType.bypass,
                    replica_groups=rg,
                    ins=[agin.opt()],
                    outs=[agout.opt()],
                )
            return ssbs, agout

        def new_st(si):
            return stp.tile([128, KT, SW], F16, name=f"st_{si}", tag="st")

        # chain
        st2 = new_st("c3")
        ss, _ = product(st1, None, fA, "c2", st_next=st2)
        st3 = new_st("c4")
        ss, _ = product(st2, ss, fA, "c3", st_next=st3)
        st4 = new_st("c8")
        ss, ag4 = product(st3, ss, fA, "c4", do_ag=True, st_next=st4)

        # filler C5 = C4 @ A' occupies the AG(C4) window and finishes st4
        ss, _ = product(st4, ss, fA, "f5")

        fC4 = fullp.tile([128, KT, G], F16, name="fC4", tag="full")
        st8 = new_st("c12")
        ss, _ = product(st4, None, fC4, "c8", reload_src=ag4, st_next=st8)
        st12 = new_st("c24")
        ss, ag12 = product(st8, ss, fC4, "c12", do_ag=True, st_next=st12)

        # filler C13 = C12 @ A' occupies the AG(C12) window and finishes st12
        ss, _ = product(st12, ss, fA, "f13")

        fC12 = fullp.tile([128, KT, G], F16, name="fC12", tag="full")
        st24 = new_st("c36")
        ss, _ = product(st12, None, fC12, "c24", reload_src=ag12, st_next=st24)
        st36 = new_st("c48")
        ss, _ = product(st24, ss, fC12, "c36", st_next=st36)
        ss, ag48 = product(st36, ss, fC12, "c48", do_ag=True)

        # u^T = C12^T @ x^T during the AG(C48) window (C12 stationary)
        ut = misc.tile([128, KT, BW], F16, name="ut", tag="ut")
        for m in range(KT):
            pu = mmps.tile([128, BW], F32, name=f"pu{m}", tag="mm")
            for k in range(KT):
                nc.tensor.matmul(
                    pu[:],
                    fC12[:, k, 128 * m : 128 * (m + 1)],
                    xsb[:, k, :],
                    start=(k == 0),
                    stop=(k == KT - 1),
                )
            nc.vector.tensor_copy(ut[:, m, :], pu[:])

        # y[Bj,:] = u[Bj,:] @ C48
        fC48 = fullp.tile([128, KT, G], F16, name="fC48", tag="full")
        for k in range(KT):
            qeng[k % 2].dma_start(fC48[:, k, :], ag48[128 * k : 128 * (k + 1), :])
        for mr in range(4):
            pts = [
                mmps.tile([128, 512], F32, name=f"py{mr}_{mc}", tag="mm")
                for mc in range(4)
            ]
            for k in range(KT):
                for mc in range(4):
                    nc.tensor.matmul(
                        pts[mc][:],
                        ut[:, k, 128 * mr : 128 * (mr + 1)],
                        fC48[:, k, 512 * mc : 512 * (mc + 1)],
                        start=(k == 0),
                        stop=(k == KT - 1),
                    )
            for mc in range(4):
                yt = ypool.tile([128, 512], F32, name=f"yt{mr}_{mc}", tag="yt")
                nc.vector.tensor_copy(yt[:], pts[mc][:])
                qeng[mc % 2].dma_start(
                    yj[128 * mr : 128 * (mr + 1), 512 * mc : 512 * (mc + 1)], yt[:]
                )
    nc.compile()
    return nc


def _prep(x, W):
    """Host prep: rescale so the fp16 chain stays O(1); fp16 casts."""
    A = np.ascontiguousarray(W.T.astype(np.float64))
    rng = np.random.default_rng(0)
    v = rng.standard_normal(_G)
    growth = []
    for _ in range(60):
        v2 = A @ v
        n2 = np.linalg.norm(v2)
        growth.append(n2 / np.linalg.norm(v))
        v = v2 / n2
    rho = float(np.exp(np.mean(np.log(growth[20:]))))
    c = 1.0 / rho
    a16 = np.ascontiguousarray((c * A).astype(np.float16))
    x16t = np.ascontiguousarray(x.astype(np.float16).T)
    return a16, x16t, c


def kernel(x, W):
    from concourse.bass_utils import run_bass_kernel_spmd

    if "nc" not in _cache:
        _cache["nc"] = _build()
    nc = _cache["nc"]

    x = np.asarray(x, dtype=np.float32)
    W = np.asarray(W, dtype=np.float32)
    a16, x16t, c = _prep(x, W)
    in_maps = [
        {
            "afull": a16,
            "slabt0": np.ascontiguousarray(a16[_SW * j : _SW * (j + 1), :].T),
            "xbj": np.ascontiguousarray(x16t[:, _BW * j : _BW * (j + 1)]),
        }
        for j in range(_NC)
    ]
    res = run_bass_kernel_spmd(nc, in_maps, core_ids=list(range(_NC)))
    _cache["last_exec_time_ns"] = res.exec_time_ns
    _cache["last_results"] = res
    scale = np.float64(c) ** -60
    y = np.concatenate(
        [res.results[j]["yj"].astype(np.float64) * scale for j in range(_NC)], axis=0
    ).astype(np.float32)
    return y
